# revision 6
# baseline (speedup 1.0000x reference)
"""IoU loss kernel for Trainium2, data-parallel over the batch dim on 8 cores.

Math (per reference):
    probs = softmax(inputs, axis=1)                       # (8, 13, 800, 800)
    intersection = sum_pix probs[b, t, h, w]
    total = probs.sum() + Npix                            # probs.sum() == Npix (+fp noise)
    out = 1 - (intersection + smooth) / (total - intersection + smooth)

Device kernel (per core, one batch item), raw Bass with manual semaphores.
Layout: pixel-partitioned (128, 13, N) chunks, class in the free dim.
Host sends x as bf16 pre-transposed so chunk DMAs are contiguous per
partition (chunks are loaded two at a time: descriptor generation, one
descriptor per partition row, is what paces HWDGE, so fewer/bigger rows
are faster), plus four u8 range-predicate masks derived from t (a pure
re-encoding of the index tensor) that drive a blocked mux tree.

Per chunk j:
  ACT : E = exp(X)  (bf16, all 13 classes)
  DVE : blocked cp tree on X in place: 13->7->4->2->1  => x_sel = X[:,0]
        each level split in two free-dim halves so the second half hides
        the predicated-write pipeline drain of the level before it
  DVE : denominator tree (dense bf16 tt, forwards without drains):
        A = E[0:6]+E[6:12]; B = A[0:3]+A[3:6]; C = (B0+B1)+(B2+E12)
  ACT : L = ln(C)
  GPS : S = x_sel - L          (off the critical path; gpsimd is erratic)
  ACT : exp(S) with accum_out -> acc[:, j]   (free per-partition reduce)
Host sums acc over cores/partitions/chunks and forms the IoU scalar.
"""

import numpy as np
import ml_dtypes

_BS, _C, _H, _W = 8, 13, 800, 800
_P = 128
_FREE = (_H * _W) // _P  # 5000
_N = 625                 # chunk free size
_NCHUNK = _FREE // _N    # 8
_NBUF = 4
_NCORES = 8
_NPIX = _BS * _H * _W    # 5120000

_cached = {}


def _build_program():
    from contextlib import ExitStack

    import concourse.bass as bass
    import concourse.mybir as mybir

    f32 = mybir.dt.float32
    bf16 = mybir.dt.bfloat16
    u8 = mybir.dt.uint8
    Alu = mybir.AluOpType
    Act = mybir.ActivationFunctionType

    nc = bass.Bass(trn_type="TRN2")
    x = nc.declare_dram_parameter("x", [_P, _NCHUNK, _C, _N], bf16,
                                  isOutput=False)
    m = nc.declare_dram_parameter("m", [_P, 4, _FREE], u8, isOutput=False)
    part = nc.declare_dram_parameter("part", [_P, _NCHUNK], f32, isOutput=True)

    ctx = ExitStack()
    with ctx:
        M = ctx.enter_context(nc.sbuf_tensor("M", [_P, 4, _FREE], u8))
        X = ctx.enter_context(nc.sbuf_tensor("X", [_P, _NBUF, _C, _N], bf16))
        E = ctx.enter_context(nc.sbuf_tensor("E", [_P, _NBUF, _C, _N], bf16))
        A = ctx.enter_context(nc.sbuf_tensor("A", [_P, 6, _N], bf16))
        B = ctx.enter_context(nc.sbuf_tensor("B", [_P, 3, _N], bf16))
        C1 = ctx.enter_context(nc.sbuf_tensor("C1", [_P, _N], bf16))
        C2 = ctx.enter_context(nc.sbuf_tensor("C2", [_P, _N], bf16))
        CD = ctx.enter_context(nc.sbuf_tensor("CD", [_P, _NBUF, _N], bf16))
        L = ctx.enter_context(nc.sbuf_tensor("L", [_P, _NBUF, _N], bf16))
        S = ctx.enter_context(nc.sbuf_tensor("S", [_P, _NBUF, _N], bf16))
        ED = ctx.enter_context(nc.sbuf_tensor("ED", [_P, _N], bf16))
        acc = ctx.enter_context(nc.sbuf_tensor("acc", [_P, _NCHUNK], f32))

        block = ctx.enter_context(nc.Block())
        dma_m = ctx.enter_context(nc.semaphore("dma_m"))
        dma_xp = [ctx.enter_context(nc.semaphore(f"dma_xp{i}"))
                  for i in range(2)]
        dma_out = ctx.enter_context(nc.semaphore("dma_out"))
        s_exp = ctx.enter_context(nc.semaphore("s_exp"))
        s_C = ctx.enter_context(nc.semaphore("s_C"))
        s_ln = ctx.enter_context(nc.semaphore("s_ln"))
        s_sub = ctx.enter_context(nc.semaphore("s_sub"))
        s_fin = ctx.enter_context(nc.semaphore("s_fin"))

        NPAIR = _NCHUNK // 2

        @block.sync
        def _(sync):
            for p in range(NPAIR):
                sl = (2 * p) % _NBUF
                if p >= 2:
                    # slots sl, sl+1 were last read by sub of chunks
                    # 2p-4, 2p-3
                    sync.wait_ge(s_sub, 2 * p - 2)
                    sync.wait_ge(dma_xp[p % 2], 16 * (p // 2))
                sync.dma_start(
                    out=X[:, sl:sl + 2, :, :], in_=x[:, 2 * p:2 * p + 2, :, :]
                ).then_inc(dma_xp[p % 2], 16)
                if p == 0:
                    # masks are first needed by the cp tree of chunk 0,
                    # which runs after exp(0)
                    sync.dma_start(out=M[:, :, :],
                                   in_=m[:, :, :]).then_inc(dma_m, 16)
            sync.wait_ge(s_fin, _NCHUNK)
            sync.dma_start(out=part[:, :], in_=acc[:, :]).then_inc(dma_out, 16)
            sync.wait_ge(dma_out, 16)

        @block.scalar
        def _(scalar):
            def ln_of(k):
                bk = k % _NBUF
                scalar.wait_ge(s_C, k + 1)
                if k >= _NBUF:
                    # L slot read by sub of chunk k-NBUF
                    scalar.wait_ge(s_sub, k - _NBUF + 1)
                scalar.activation(
                    out=L[:, bk, :], in_=CD[:, bk, :], func=Act.Ln
                ).then_inc(s_ln, 1)

            def expacc_of(k):
                bk = k % _NBUF
                scalar.wait_ge(s_sub, k + 1)
                scalar.activation(
                    out=ED[:, :], in_=S[:, bk, :], func=Act.Exp,
                    accum_out=acc[:, k:k + 1],
                ).then_inc(s_fin, 1)

            for j in range(_NCHUNK):
                b = j % _NBUF
                p = j // 2
                scalar.wait_ge(dma_xp[p % 2], 16 * (p // 2 + 1))
                if j >= _NBUF:
                    # E slot fully consumed by C2 of chunk j-NBUF
                    scalar.wait_ge(s_C, j - _NBUF + 1)
                scalar.activation(
                    out=E[:, b, :, :], in_=X[:, b, :, :], func=Act.Exp
                ).then_inc(s_exp, 1)
                if j >= 1:
                    ln_of(j - 1)
                if j >= 2:
                    expacc_of(j - 2)
            ln_of(_NCHUNK - 1)
            expacc_of(_NCHUNK - 2)
            expacc_of(_NCHUNK - 1)

        @block.gpsimd
        def _(gpsimd):
            for j in range(_NCHUNK):
                b = j % _NBUF
                gpsimd.wait_ge(s_ln, j + 1)
                if j >= _NBUF:
                    # S slot read by expacc of chunk j-NBUF
                    gpsimd.wait_ge(s_fin, j - _NBUF + 1)
                gpsimd.tensor_tensor(
                    out=S[:, b, :], in0=X[:, b, 0, :], in1=L[:, b, :],
                    op=Alu.subtract,
                ).then_inc(s_sub, 1)

        @block.vector
        def _(vector):
            vector.wait_ge(dma_m, 16)
            for j in range(_NCHUNK):
                b = j % _NBUF
                vector.wait_ge(s_exp, j + 1)
                # denominator lvl1: A = E[:,0:6]+E[:,6:12]
                vector.tensor_tensor(
                    out=A[:, :, :], in0=E[:, b, 0:6, :],
                    in1=E[:, b, 6:12, :], op=Alu.add)
                # blocked mux tree on X (in place): 13 -> 7 -> 4 -> 2 -> 1,
                # levels split into free-dim halves to hide the
                # predicated-write drain between dependent levels
                NH = _N // 2
                tree = [(0, 6, 7), (1, 3, 4), (2, 2, 2), (3, 1, 1)]
                for lev, width, off in tree:
                    for h in range(2):
                        fsl = slice(h * NH, (h + 1) * NH)
                        msl = slice(j * _N + h * NH, j * _N + (h + 1) * NH)
                        mk = M[:, lev, msl].unsqueeze(1)
                        vector.copy_predicated(
                            X[:, b, 0:width, fsl],
                            mk.broadcast_to((_P, width, NH)),
                            X[:, b, off:off + width, fsl])
                # denominator lvl2 + lvl3 (dense tt forwards, no drain)
                if j >= _NBUF:
                    # CD slot read by ln of chunk j-NBUF
                    vector.wait_ge(s_ln, j - _NBUF + 1)
                vector.tensor_tensor(out=B[:, :, :], in0=A[:, 0:3, :],
                                     in1=A[:, 3:6, :], op=Alu.add)
                vector.tensor_tensor(out=C1[:, :], in0=B[:, 0, :],
                                     in1=B[:, 1, :], op=Alu.add)
                vector.tensor_tensor(out=C2[:, :], in0=B[:, 2, :],
                                     in1=E[:, b, 12, :], op=Alu.add)
                vector.tensor_tensor(
                    out=CD[:, b, :], in0=C1[:, :], in1=C2[:, :], op=Alu.add,
                ).then_inc(s_C, 1)

    return nc


def _get_program():
    if "nc" not in _cached:
        _cached["nc"] = _build_program()
    return _cached["nc"]


def _make_in_maps(inputs, targets):
    in_maps = []
    for b in range(_NCORES):
        xb = np.asarray(inputs[b]).reshape(_C, _P, _FREE)
        # (128, NCHUNK, 13, N) so each chunk is contiguous per partition
        xh = np.ascontiguousarray(
            xb.transpose(1, 0, 2).reshape(_P, _C, _NCHUNK, _N)
            .transpose(0, 2, 1, 3)
        ).astype(ml_dtypes.bfloat16)
        t = np.asarray(targets[b]).astype(np.int64).reshape(_P, _FREE)
        # blocked mux-tree predicates (pure index re-encoding of t)
        ma = t >= 7
        t1 = t - 7 * ma
        mb = t1 >= 4
        t2 = t1 - 4 * mb
        mc = t2 >= 2
        t3 = t2 - 2 * mc
        md = t3 >= 1
        mh = np.ascontiguousarray(
            np.stack([ma, mb, mc, md], axis=1).astype(np.uint8))
        in_maps.append({"x": xh, "m": mh})
    return in_maps


def _finalize(parts, smooth):
    inter = 0.0
    for p in parts:
        inter += float(np.sum(np.asarray(p).astype(np.float64)))
    s = float(smooth)
    total = 2.0 * float(_NPIX)
    union = total - inter
    out = 1.0 - (inter + s) / (union + s)
    return np.asarray(np.float32(out))


def kernel(inputs, targets, smooth):
    from concourse.bass_utils import run_bass_kernel_spmd

    nc = _get_program()
    in_maps = _make_in_maps(np.asarray(inputs), np.asarray(targets))
    res = run_bass_kernel_spmd(nc, in_maps, list(range(_NCORES)))
    return _finalize([res.results[b]["part"] for b in range(_NCORES)], smooth)


# revision 10
# speedup vs baseline: 1.2888x; 1.2888x over previous
"""IoU loss kernel for Trainium2, data-parallel over the batch dim on 8 cores.

Math (per reference):
    probs = softmax(inputs, axis=1)                       # (8, 13, 800, 800)
    intersection = sum_pix probs[b, t, h, w]
    total = probs.sum() + Npix                            # probs.sum() == Npix (+fp noise)
    out = 1 - (intersection + smooth) / (total - intersection + smooth)

Device kernel (per core, one batch item), raw Bass with manual semaphores.
Layout: pixel-partitioned (128, 13, N) chunks, class in the free dim.
Host sends x as bf16 pre-transposed so chunk DMAs are contiguous per
partition (chunks are loaded two at a time: descriptor generation, one
descriptor per partition row, is what paces HWDGE, so fewer/bigger rows
are faster), plus four u8 range-predicate masks derived from t (a pure
re-encoding of the index tensor) that drive a blocked mux tree.

Per chunk j:
  ACT : E = exp(X)  (bf16, all 13 classes)
  DVE : blocked cp tree on X in place: 13->7->4->2->1  => x_sel = X[:,0]
        each level split in two free-dim halves so the second half hides
        the predicated-write pipeline drain of the level before it
  DVE : denominator tree (dense bf16 tt, forwards without drains):
        A = E[0:6]+E[6:12]; B = A[0:3]+A[3:6]; C = (B0+B1)+(B2+E12)
  ACT : L = ln(C)
  GPS : S = x_sel - L          (off the critical path; gpsimd is erratic)
  ACT : exp(S) with accum_out -> acc[:, j]   (free per-partition reduce)
Host sums acc over cores/partitions/chunks and forms the IoU scalar.
"""

import numpy as np
import ml_dtypes

_BS, _C, _H, _W = 8, 13, 800, 800
_P = 128
_FREE = (_H * _W) // _P  # 5000
_N = 625                 # chunk free size
_NCHUNK = _FREE // _N    # 8
_NBUF = 4
_NCORES = 8
_NPIX = _BS * _H * _W    # 5120000

_cached = {}


def _build_program():
    from contextlib import ExitStack

    import concourse.bass as bass
    import concourse.mybir as mybir

    f32 = mybir.dt.float32
    bf16 = mybir.dt.bfloat16
    u8 = mybir.dt.uint8
    Alu = mybir.AluOpType
    Act = mybir.ActivationFunctionType

    nc = bass.Bass(trn_type="TRN2")
    x = nc.declare_dram_parameter("x", [_P, _NCHUNK, _C, _N], bf16,
                                  isOutput=False)
    m = nc.declare_dram_parameter("m", [_P, 4, _FREE], u8, isOutput=False)
    part = nc.declare_dram_parameter("part", [_P, _NCHUNK], f32, isOutput=True)

    ctx = ExitStack()
    with ctx:
        M = ctx.enter_context(nc.sbuf_tensor("M", [_P, 4, _FREE], u8))
        X = ctx.enter_context(nc.sbuf_tensor("X", [_P, _NBUF, _C, _N], bf16))
        E = ctx.enter_context(nc.sbuf_tensor("E", [_P, _NBUF, _C, _N], bf16))
        A = ctx.enter_context(nc.sbuf_tensor("A", [_P, 6, _N], bf16))
        B = ctx.enter_context(nc.sbuf_tensor("B", [_P, 3, _N], bf16))
        C1 = ctx.enter_context(nc.sbuf_tensor("C1", [_P, _N], bf16))
        C2 = ctx.enter_context(nc.sbuf_tensor("C2", [_P, _N], bf16))
        CD = ctx.enter_context(nc.sbuf_tensor("CD", [_P, _NBUF, _N], bf16))
        L = ctx.enter_context(nc.sbuf_tensor("L", [_P, _NBUF, _N], bf16))
        S = ctx.enter_context(nc.sbuf_tensor("S", [_P, _NBUF, _N], bf16))
        ED = ctx.enter_context(nc.sbuf_tensor("ED", [_P, _N], bf16))
        acc = ctx.enter_context(nc.sbuf_tensor("acc", [_P, _NCHUNK], f32))

        block = ctx.enter_context(nc.Block())
        dma_m = ctx.enter_context(nc.semaphore("dma_m"))
        dma_x0a = ctx.enter_context(nc.semaphore("dma_x0a"))
        dma_xc = [ctx.enter_context(nc.semaphore(f"dma_xc{i}"))
                  for i in range(_NBUF)]
        dma_out = ctx.enter_context(nc.semaphore("dma_out"))
        s_exp = ctx.enter_context(nc.semaphore("s_exp"))
        s_C = ctx.enter_context(nc.semaphore("s_C"))
        s_ln = ctx.enter_context(nc.semaphore("s_ln"))
        s_sub = ctx.enter_context(nc.semaphore("s_sub"))
        s_fin = ctx.enter_context(nc.semaphore("s_fin"))

        HM = _FREE // 2

        @block.sync
        def _(sync):
            # HWDGE is descriptor-generation bound (one descriptor per
            # partition row), so the ramp is paced by descriptor count,
            # not bytes: split chunk 0 and the masks into halves so the
            # scalar engine can start exp as early as possible.
            sync.dma_start(out=X[:, 0, 0:7, :],
                           in_=x[:, 0, 0:7, :]).then_inc(dma_x0a, 16)
            sync.dma_start(out=X[:, 0, 7:13, :],
                           in_=x[:, 0, 7:13, :]).then_inc(dma_xc[0], 16)
            sync.dma_start(out=M[:, :, 0:HM],
                           in_=m[:, :, 0:HM]).then_inc(dma_m, 16)
            for j in range(1, _NCHUNK):
                b = j % _NBUF
                if j >= _NBUF:
                    # X slot b last read by sub of chunk j-NBUF
                    sync.wait_ge(s_sub, j - _NBUF + 1)
                    sync.wait_ge(dma_xc[b], 16 * (j // _NBUF))
                sync.dma_start(
                    out=X[:, b, :, :], in_=x[:, j, :, :]
                ).then_inc(dma_xc[b], 16)
                if j == 2:
                    sync.dma_start(out=M[:, :, HM:_FREE],
                                   in_=m[:, :, HM:_FREE]).then_inc(dma_m, 16)
            sync.wait_ge(s_fin, _NCHUNK)
            sync.dma_start(out=part[:, :], in_=acc[:, :]).then_inc(dma_out, 16)
            sync.wait_ge(dma_out, 16)

        @block.scalar
        def _(scalar):
            def ln_of(k):
                bk = k % _NBUF
                scalar.wait_ge(s_C, k + 1)
                if k >= _NBUF:
                    # L slot read by sub of chunk k-NBUF
                    scalar.wait_ge(s_sub, k - _NBUF + 1)
                scalar.activation(
                    out=L[:, bk, :], in_=CD[:, bk, :], func=Act.Ln
                ).then_inc(s_ln, 1)

            def expacc_of(k):
                bk = k % _NBUF
                scalar.wait_ge(s_sub, k + 1)
                scalar.activation(
                    out=ED[:, :], in_=S[:, bk, :], func=Act.Exp,
                    accum_out=acc[:, k:k + 1],
                ).then_inc(s_fin, 1)

            for j in range(_NCHUNK):
                b = j % _NBUF
                if j == 0:
                    # chunk 0 arrives in two class-halves; exp each as
                    # soon as it lands
                    scalar.wait_ge(dma_x0a, 16)
                    scalar.activation(out=E[:, 0, 0:7, :],
                                      in_=X[:, 0, 0:7, :], func=Act.Exp)
                    scalar.wait_ge(dma_xc[0], 16)
                    scalar.activation(
                        out=E[:, 0, 7:13, :], in_=X[:, 0, 7:13, :],
                        func=Act.Exp,
                    ).then_inc(s_exp, 1)
                    continue
                scalar.wait_ge(dma_xc[b], 16 * (j // _NBUF + 1))
                if j >= _NBUF:
                    # E slot fully consumed by C2 of chunk j-NBUF
                    scalar.wait_ge(s_C, j - _NBUF + 1)
                scalar.activation(
                    out=E[:, b, :, :], in_=X[:, b, :, :], func=Act.Exp
                ).then_inc(s_exp, 1)
                if j >= 1:
                    ln_of(j - 1)
                if j >= 2:
                    expacc_of(j - 2)
            ln_of(_NCHUNK - 1)
            expacc_of(_NCHUNK - 2)
            expacc_of(_NCHUNK - 1)

        @block.vector
        def _(vector):
            def sub_of(k):
                bk = k % _NBUF
                vector.wait_ge(s_ln, k + 1)
                if k >= _NBUF:
                    # S slot read by expacc of chunk k-NBUF
                    vector.wait_ge(s_fin, k - _NBUF + 1)
                vector.tensor_tensor(
                    out=S[:, bk, :], in0=X[:, bk, 0, :], in1=L[:, bk, :],
                    op=Alu.subtract,
                ).then_inc(s_sub, 1)

            vector.wait_ge(dma_m, 16)
            for j in range(_NCHUNK):
                b = j % _NBUF
                if j == _NCHUNK // 2:
                    vector.wait_ge(dma_m, 32)
                vector.wait_ge(s_exp, j + 1)
                # denominator lvl1: A = E[:,0:6]+E[:,6:12]
                vector.tensor_tensor(
                    out=A[:, :, :], in0=E[:, b, 0:6, :],
                    in1=E[:, b, 6:12, :], op=Alu.add)
                # blocked mux tree on X (in place): 13 -> 7 -> 4 -> 2 -> 1,
                # levels split into free-dim halves to hide the
                # predicated-write drain between dependent levels
                NH = _N // 2
                tree = [(0, 6, 7), (1, 3, 4), (2, 2, 2), (3, 1, 1)]
                for lev, width, off in tree:
                    for h in range(2):
                        fsl = slice(h * NH, (h + 1) * NH)
                        msl = slice(j * _N + h * NH, j * _N + (h + 1) * NH)
                        mk = M[:, lev, msl].unsqueeze(1)
                        vector.copy_predicated(
                            X[:, b, 0:width, fsl],
                            mk.broadcast_to((_P, width, NH)),
                            X[:, b, off:off + width, fsl])
                # denominator lvl2 + lvl3 (dense tt forwards, no drain)
                if j >= _NBUF:
                    # CD slot read by ln of chunk j-NBUF
                    vector.wait_ge(s_ln, j - _NBUF + 1)
                vector.tensor_tensor(out=B[:, :, :], in0=A[:, 0:3, :],
                                     in1=A[:, 3:6, :], op=Alu.add)
                vector.tensor_tensor(out=C1[:, :], in0=B[:, 0, :],
                                     in1=B[:, 1, :], op=Alu.add)
                vector.tensor_tensor(out=C2[:, :], in0=B[:, 2, :],
                                     in1=E[:, b, 12, :], op=Alu.add)
                vector.tensor_tensor(
                    out=CD[:, b, :], in0=C1[:, :], in1=C2[:, :], op=Alu.add,
                ).then_inc(s_C, 1)
                if j >= 1:
                    sub_of(j - 1)
            sub_of(_NCHUNK - 1)

    return nc


def _get_program():
    if "nc" not in _cached:
        _cached["nc"] = _build_program()
    return _cached["nc"]


def _make_in_maps(inputs, targets):
    in_maps = []
    for b in range(_NCORES):
        xb = np.asarray(inputs[b]).reshape(_C, _P, _FREE)
        # (128, NCHUNK, 13, N) so each chunk is contiguous per partition
        xh = np.ascontiguousarray(
            xb.transpose(1, 0, 2).reshape(_P, _C, _NCHUNK, _N)
            .transpose(0, 2, 1, 3)
        ).astype(ml_dtypes.bfloat16)
        t = np.asarray(targets[b]).astype(np.int64).reshape(_P, _FREE)
        # blocked mux-tree predicates (pure index re-encoding of t)
        ma = t >= 7
        t1 = t - 7 * ma
        mb = t1 >= 4
        t2 = t1 - 4 * mb
        mc = t2 >= 2
        t3 = t2 - 2 * mc
        md = t3 >= 1
        mh = np.ascontiguousarray(
            np.stack([ma, mb, mc, md], axis=1).astype(np.uint8))
        in_maps.append({"x": xh, "m": mh})
    return in_maps


def _finalize(parts, smooth):
    inter = 0.0
    for p in parts:
        inter += float(np.sum(np.asarray(p).astype(np.float64)))
    s = float(smooth)
    total = 2.0 * float(_NPIX)
    union = total - inter
    out = 1.0 - (inter + s) / (union + s)
    return np.asarray(np.float32(out))


def kernel(inputs, targets, smooth):
    from concourse.bass_utils import run_bass_kernel_spmd

    nc = _get_program()
    in_maps = _make_in_maps(np.asarray(inputs), np.asarray(targets))
    res = run_bass_kernel_spmd(nc, in_maps, list(range(_NCORES)))
    return _finalize([res.results[b]["part"] for b in range(_NCORES)], smooth)
